# revision 1
# baseline (speedup 1.0000x reference)
"""Trainium2 Bass kernel for nn_Attention (topk_masking).

reference:
    h = tanh(x @ W1 + b1); e = h @ W2 + b2            # [B,T,1]
    thr = sort(e, axis=1)[:, T//2]                    # per-sample median-index value
    mask: keep e < thr; softmax over kept; out = sum_t beta_t * x_t  -> [B,D,1,1]

Sharding: B=32 across 8 cores (4 samples/core), fully data-parallel.

Per-core pipeline:
  pass1: hT = tanh(W1^T x^T + b1) via fp32 matmuls (xT streamed from DRAM),
         e = W2^T hT (fp32 matmuls, M=1), e rows bounced through DRAM.
  bisect: batched over 4 samples on an E[128,128] relayout; 35 iterations of
          count(e < mid) vs 2048, then exact theta = min{e >= lo} so the kept
          set matches sort()[2048] bit-exactly.
  softmax: beta = exp(e - theta) * [e < theta] / Z  (masked to -1e8 pre-exp).
  pass2: out[d] = sum_t beta_t x[t,d] on VectorE via tensor_tensor_reduce over
         a bf16 copy of xT (beta broadcast across partitions by GpSimd).

b2 is dropped: it shifts e and thr equally and softmax is shift-invariant.
"""
import os
import sys

sys.path.insert(0, "/opt/trn_rl_repo")

import numpy as np
import ml_dtypes

import concourse.bass as bass  # noqa: F401
from concourse import bacc
import concourse.tile as tile
import concourse.mybir as mybir
from concourse.bass_utils import run_bass_kernel_spmd

F32 = mybir.dt.float32
BF16 = mybir.dt.bfloat16
U8 = mybir.dt.uint8
AF = mybir.ActivationFunctionType
ALU = mybir.AluOpType
AX = mybir.AxisListType

BSH, T, D, H = 4, 4096, 1024, 256
TT = 512  # pass1 T-tile
NEG_BIG = -99999999.0
N_ITER = int(os.environ.get("K_NITER", "33"))
PHASE = int(os.environ.get("K_PHASE", "4"))  # 1=p1, 2=+bisect, 3=+softmax, 4=full


def build(repeat=1):
    nc = bacc.Bacc(trn_type="TRN2", target_bir_lowering=False)

    xTb = nc.declare_dram_parameter("xTb", [BSH, 128, 8, T], BF16, isOutput=False)
    xTl = nc.declare_dram_parameter("xTl", [BSH, 128, 8, T], BF16, isOutput=False)
    w1sh = nc.declare_dram_parameter("w1sh", [128, 8, H], BF16, isOutput=False)
    w1sl = nc.declare_dram_parameter("w1sl", [128, 8, H], BF16, isOutput=False)
    b1s = nc.declare_dram_parameter("b1s", [128, 2], F32, isOutput=False)
    w2s = nc.declare_dram_parameter("w2s", [128, 2], F32, isOutput=False)
    out = nc.declare_dram_parameter("out", [BSH, 8, 128], F32, isOutput=True)

    with tile.TileContext(nc) as tc:
        with tc.tile_pool(name="w", bufs=1) as wpool, \
             tc.tile_pool(name="x", bufs=4) as xpool, \
             tc.tile_pool(name="h", bufs=4) as hpool, \
             tc.tile_pool(name="e", bufs=1) as epool, \
             tc.tile_pool(name="bis", bufs=1) as bpool, \
             tc.tile_pool(name="p2", bufs=3) as p2pool, \
             tc.tile_pool(name="ps", bufs=4, space="PSUM") as pspool, \
             tc.tile_pool(name="pse", bufs=4, space="PSUM") as psepool, \
             tc.tile_pool(name="dram", bufs=1, space="DRAM") as dpool:

            e_dram = dpool.tile([BSH, T], F32, tag="e_dram")
            w1h_sb = wpool.tile([128, 8, H], BF16, tag="w1h")
            nc.sync.dma_start(w1h_sb[:], w1sh.ap())
            w1l_sb = wpool.tile([128, 8, H], BF16, tag="w1l")
            nc.sync.dma_start(w1l_sb[:], w1sl.ap())
            b1_sb = wpool.tile([128, 2], F32, tag="b1")
            nc.sync.dma_start(b1_sb[:], b1s.ap())
            w2_sb = wpool.tile([128, 2], F32, tag="w2")
            nc.sync.dma_start(w2_sb[:], w2s.ap())

            rep_ctx = tc.For_i(0, repeat, 1) if repeat > 1 else None
            import contextlib
            with (rep_ctx if rep_ctx is not None else contextlib.nullcontext()):
                # Per-sample pipeline: pass1(b) -> bisect(b) -> softmax(b)
                # -> pass2(b), with sample b's post-processing overlapping
                # pass1(b+1) (Tile schedules by dependency).
                nbig4 = epool.tile([128, T], F32, tag="nbig4")
                nc.vector.memset(nbig4[:], NEG_BIG)
                e_all4 = epool.tile([128, T], F32, tag="e_all4")
                u4 = epool.tile([128, T], F32, tag="u4")
                m4 = epool.tile([128, T], U8, tag="m4")
                beta4 = epool.tile([128, T], BF16, tag="beta4")
                tp4 = bpool.tile([128, 1], F32, tag="tp4")
                tn4 = bpool.tile([128, 1], F32, tag="tn4")
                z4 = bpool.tile([128, 1], F32, tag="z4")
                rz4 = bpool.tile([128, 1], F32, tag="rz4")

                def emit_p1(b):
                    # ---------------- pass 1 (sample b) ----------------
                    for ti in range(T // TT):
                        sl = slice(ti * TT, (ti + 1) * TT)
                        xh = xpool.tile([128, 8, TT], BF16, tag="xh")
                        nc.sync.dma_start(xh[:], xTb.ap()[b, :, :, sl])
                        xl = xpool.tile([128, 8, TT], BF16, tag="xl")
                        nc.sync.dma_start(xl[:], xTl.ap()[b, :, :, sl])
                        hs = []
                        for hh in range(2):
                            hsl = slice(hh * 128, (hh + 1) * 128)
                            ps = pspool.tile([128, TT], F32, tag="hps")
                            for dc in range(8):
                                nc.tensor.matmul(
                                    ps[:], w1h_sb[:, dc, hsl], xh[:, dc, :],
                                    start=(dc == 0), stop=False,
                                )
                                nc.tensor.matmul(
                                    ps[:], w1h_sb[:, dc, hsl], xl[:, dc, :],
                                    start=False, stop=False,
                                )
                                nc.tensor.matmul(
                                    ps[:], w1l_sb[:, dc, hsl], xh[:, dc, :],
                                    start=False, stop=(dc == 7),
                                )
                            hsb = hpool.tile([128, TT], F32, tag="h")
                            nc.scalar.activation(
                                hsb[:], ps[:], AF.Tanh, bias=b1_sb[:, hh : hh + 1]
                            )
                            hs.append(hsb)
                        eps = psepool.tile([1, TT], F32, tag="eps")
                        nc.tensor.matmul(eps[:], w2_sb[:, 0:1], hs[0][:], start=True, stop=False)
                        nc.tensor.matmul(eps[:], w2_sb[:, 1:2], hs[1][:], start=False, stop=True)
                        estage = hpool.tile([1, TT], F32, tag="estage")
                        nc.scalar.copy(estage[:], eps[:])
                        nc.sync.dma_start(e_dram[b : b + 1, sl], estage[:])

                def emit_chain(g):
                    if PHASE < 2:
                        return None
                    # bisection for samples 2g, 2g+1 on a [64,128] relayout,
                    # pure-DVE chain (transpose-reduce + stream_shuffle)
                    Eb = bpool.tile([64, 128], F32, tag="Eb", bufs=2, name=f"Eb{g}")
                    for j in range(2):
                        b = 2 * g + j
                        nc.sync.dma_start(
                            Eb[32 * j : 32 * j + 32, :],
                            e_dram[b].rearrange("(lp f) -> lp f", lp=32),
                        )
                    BCAST0 = [0] * 32
                    lo = bpool.tile([64, 1], F32, tag="lo", bufs=2, name=f"lo{g}")
                    hi = bpool.tile([64, 1], F32, tag="hi", bufs=2, name=f"hi{g}")
                    nc.vector.memset(lo[:], -17.0)
                    nc.vector.memset(hi[:], 17.0)
                    mid = bpool.tile([64, 1], F32, tag="mid", bufs=2, name=f"mid{g}")
                    cmp_t = bpool.tile([64, 128], U8, tag="cmp", bufs=2, name=f"cmp{g}")
                    cscr = bpool.tile([64, 32], F32, tag="cscr", bufs=2, name=f"cscr{g}")
                    nc.vector.memset(cscr[:], 0.0)
                    tot = bpool.tile([64, 1], F32, tag="tot", bufs=2, name=f"tot{g}")
                    totb = bpool.tile([64, 1], F32, tag="totb", bufs=2, name=f"totb{g}")
                    msk = bpool.tile([64, 1], U8, tag="msk", bufs=2, name=f"msk{g}")
                    for _ in range(N_ITER):
                        nc.vector.tensor_scalar(mid[:], lo[:], hi[:], 0.5, ALU.add, ALU.mult)
                        nc.vector.tensor_scalar(
                            cmp_t[:], Eb[:], mid[:], 0.0, ALU.is_lt, ALU.add,
                            accum_out=cscr[:, 0:1],
                        )
                        nc.vector.tensor_reduce(
                            tot[:], cscr[:], axis=AX.X, op=ALU.add, apply_transpose=True
                        )
                        nc.vector.stream_shuffle(totb[:], tot[:], BCAST0)
                        nc.vector.tensor_scalar(msk[:], totb[:], 2048.5, None, ALU.is_lt)
                        nc.vector.copy_predicated(lo[:], msk[:], mid[:])
                        nc.vector.tensor_scalar(msk[:], totb[:], 2048.5, None, ALU.is_ge)
                        nc.vector.copy_predicated(hi[:], msk[:], mid[:])
                    return lo

                def emit_post(g, lo):
                    if PHASE < 3:
                        return
                    for j in range(2):
                        b = 2 * g + j
                        # ------------- softmax (sample b) -------------
                        nc.sync.dma_start(tp4[32 * b : 32 * b + 1, :], lo[32 * j : 32 * j + 1, :])
                        nc.sync.dma_start(e_all4[32 * b : 32 * b + 1, :], e_dram[b : b + 1, :])
                        nc.vector.tensor_scalar(
                            tn4[32 * b : 32 * b + 1, :], tp4[32 * b : 32 * b + 1, :], -1.0, None, ALU.mult
                        )
                        nc.vector.tensor_scalar(
                            m4[32 * b : 32 * b + 1, :], e_all4[32 * b : 32 * b + 1, :],
                            tp4[32 * b : 32 * b + 1, :], None, ALU.is_ge,
                        )
                        nc.vector.copy_predicated(
                            e_all4[32 * b : 32 * b + 1, :], m4[32 * b : 32 * b + 1, :],
                            nbig4[32 * b : 32 * b + 1, :],
                        )
                        nc.scalar.activation(
                            u4[32 * b : 32 * b + 1, :], e_all4[32 * b : 32 * b + 1, :], AF.Exp,
                            bias=tn4[32 * b : 32 * b + 1, :], scale=1.0,
                            accum_out=z4[32 * b : 32 * b + 1, :],
                        )
                        nc.vector.reciprocal(rz4[32 * b : 32 * b + 1, :], z4[32 * b : 32 * b + 1, :])
                        nc.vector.tensor_scalar(
                            beta4[32 * b : 32 * b + 1, :], u4[32 * b : 32 * b + 1, :],
                            rz4[32 * b : 32 * b + 1, :], None, ALU.mult,
                        )
                        if PHASE < 4:
                            continue
                        # ------------- pass 2 (sample b) -------------
                        accs = p2pool.tile([128, 8], F32, tag=f"acc{b}", bufs=1,
                                           name=f"accs{b}")
                        nc.vector.memset(accs[:], 0.0)
                        brow = epool.tile([1, T], BF16, tag="brow", bufs=2, name=f"brow{b}")
                        nc.sync.dma_start(brow[:], beta4[32 * b : 32 * b + 1, :])
                        for ti in range(T // TT):
                            sl = slice(ti * TT, (ti + 1) * TT)
                            ub = p2pool.tile([128, 1, TT], BF16, tag="ub")
                            nc.gpsimd.partition_broadcast(
                                ub[:, 0, :], brow[:, sl], channels=128
                            )
                            xb = p2pool.tile([128, 8, TT], BF16, tag="xb")
                            nc.sync.dma_start(xb[:], xTb.ap()[b, :, :, sl])
                            nc.vector.tensor_tensor(
                                out=xb[:], in0=xb[:],
                                in1=ub[:].broadcast_to([128, 8, TT]), op=ALU.mult,
                            )
                            cur = p2pool.tile([128, 8], F32, tag="cur")
                            junk = p2pool.tile([128, TT], BF16, tag="junk")
                            # balance the 8 chunk-reductions: 5 on ACT, 3 on DVE
                            for dc in range(5):
                                nc.scalar.activation(
                                    junk[:], xb[:, dc, :], AF.Copy,
                                    accum_out=cur[:, dc : dc + 1],
                                )
                            nc.vector.tensor_reduce(
                                cur[:, 5:8], xb[:, 5:8, :], axis=AX.X, op=ALU.add
                            )
                            nc.vector.tensor_tensor(
                                out=accs[:], in0=accs[:], in1=cur[:], op=ALU.add
                            )
                        for dc in range(8):
                            nc.sync.dma_start(out.ap()[b, dc, :], accs[:, dc : dc + 1])

                emit_p1(0)
                emit_p1(1)
                lo0 = emit_chain(0)
                emit_p1(2)
                emit_post(0, lo0)
                emit_p1(3)
                lo1 = emit_chain(1)
                emit_post(1, lo1)
                if PHASE < 4:
                    zt = p2pool.tile([128, 8], F32, tag="zt")
                    nc.vector.memset(zt[:], float(PHASE))
                    for b in range(BSH):
                        for dc in range(8):
                            nc.sync.dma_start(out.ap()[b, dc, :], zt[:, dc : dc + 1])


    nc.finalize()
    return nc


_NC_CACHE = None


def _get_nc():
    global _NC_CACHE
    if _NC_CACHE is None:
        _NC_CACHE = build()
    return _NC_CACHE


def make_in_maps(x, W1, b1, W2, b2):
    del b2  # shift-invariant: no effect on the output
    x = np.asarray(x, dtype=np.float32)
    W1 = np.asarray(W1, dtype=np.float32)
    b1 = np.asarray(b1, dtype=np.float32).reshape(H)
    W2 = np.asarray(W2, dtype=np.float32).reshape(H)

    w1r = np.ascontiguousarray(W1.reshape(8, 128, H).transpose(1, 0, 2))
    w1sh = w1r.astype(ml_dtypes.bfloat16)
    w1sl = (w1r - w1sh.astype(np.float32)).astype(ml_dtypes.bfloat16)
    b1s = np.ascontiguousarray(b1.reshape(2, 128).T)
    w2s = np.ascontiguousarray(W2.reshape(2, 128).T)

    in_maps = []
    for c in range(8):
        xs = x[4 * c : 4 * c + 4]  # [4, T, D]
        xt = np.ascontiguousarray(
            xs.transpose(0, 2, 1).reshape(BSH, 8, 128, T).transpose(0, 2, 1, 3)
        )  # [4, 128, 8, T]; xt[b,p,dc,t] = x[b,t,dc*128+p]
        xh = xt.astype(ml_dtypes.bfloat16)
        xlo = (xt - xh.astype(np.float32)).astype(ml_dtypes.bfloat16)
        in_maps.append(
            {
                "xTb": xh,
                "xTl": xlo,
                "w1sh": w1sh,
                "w1sl": w1sl,
                "b1s": b1s,
                "w2s": w2s,
            }
        )
    return in_maps


def kernel(x, W1, b1, W2, b2):
    nc = _get_nc()
    in_maps = make_in_maps(x, W1, b1, W2, b2)
    res = run_bass_kernel_spmd(nc, in_maps, core_ids=list(range(8)))
    outs = [res.results[c]["out"].reshape(BSH, 1024) for c in range(8)]
    full = np.concatenate(outs, axis=0).astype(np.float32)  # [32, 1024]
    return full[:, :, None, None]



# revision 21
# speedup vs baseline: 1.3757x; 1.3757x over previous
"""Trainium2 Bass kernel for nn_Attention (topk_masking), v2.

reference:
    h = tanh(x @ W1 + b1); e = h @ W2 + b2            # [B,T,1]
    thr = sort(e, axis=1)[:, T//2]                    # per-sample threshold
    mask: keep e < thr; softmax over kept; out = sum_t beta_t x_t -> [B,D,1,1]

Sharding: B=32 across 8 cores (4 samples/core), fully data-parallel.

v2 design (vs v1's 3-matmul bf16 hi/lo split):
  - pass1 runs a SINGLE fp16 matmul chain (x, W1, W2, h all fp16, fp32 PSUM
    accum). Verified in numpy: rel err 0.016 < 2e-2 gate (threshold-boundary
    flips dominate; bf16-only would be 0.041).
  - x is loaded ONCE per sample (fp16 xT layout, quarter-T tiles) and kept
    resident in SBUF through pass2 -> DMA traffic 33.6MB/core instead of 100MB.
  - threshold via per-sample 4-ary bisection (12 rounds, w_final 4.8e-7) on a
    [32,128] relayout of e; exact count<=2048 invariant in fp32.  Solo chains
    start right after each sample's pass1 so only the last one is exposed.
  - softmax batched per sample on [32,128] (ACT exp with accum).
  - pass2 out[d] = sum_t beta_t xT[d,t] split across engines per (dc, quarter)
    chunk: Pool scalar_tensor_tensor (mult+accum 1 instr), DVE
    tensor_tensor_reduce, and DVE-mult + ACT-copy-accum, balancing busy time.
b2 is dropped: it shifts e and thr equally; softmax is shift-invariant.
"""
import os
import sys
import contextlib

sys.path.insert(0, "/opt/trn_rl_repo")

import numpy as np
import ml_dtypes

import concourse.bass as bass  # noqa: F401
from concourse import bass_isa
from concourse import bacc
import concourse.tile as tile
import concourse.mybir as mybir
from concourse.bass_utils import run_bass_kernel_spmd

F32 = mybir.dt.float32
F16 = mybir.dt.float16
U8 = mybir.dt.uint8
AF = mybir.ActivationFunctionType
ALU = mybir.AluOpType
AX = mybir.AxisListType

BSH, T, D, H = 4, 4096, 1024, 256
TT = 512          # pass1 T-tile
NQ = 4            # x resident tiles per sample (quarter-T granularity)
TQ = T // NQ      # 1024
NEG_BIG = -99999999.0
N_ITER = int(os.environ.get("K_NITER", "10"))   # 4-ary: 0.5*4^-10 ~ 5e-7
PHASE = int(os.environ.get("K_PHASE", "4"))     # 1=p1, 2=+bisect, 3=+softmax, 4=full
XBUFS = int(os.environ.get("K_XBUFS", "10"))
BCBUFS = int(os.environ.get("K_BCBUFS", "1"))
BCAST0 = [0] * 32

# pass2 chunk -> engine map per (b, dc).
#   B = DVE multiply + ACT copy-accum reduce
#   C = Pool multiply + ACT copy-accum reduce
#   D = Pool multiply + DVE reduce
#   E = DVE multiply + DVE reduce
# (tensor_scalar/STT are NOT legal on Pool; tensor_tensor_reduce crashes the
# device on TRN2 hardware - avoid both)
ENGMAP = [
    "BBBBCCDE",  # b=0
    "BBBCCDDE",  # b=1
    "BBBBCCDE",  # b=2
    "BBBCDDEE",  # b=3
]


def build(repeat=1):
    nc = bacc.Bacc(trn_type="TRN2", target_bir_lowering=False)

    # xt[b, q, p, dc, tq] = x[4c+b, q*TQ+tq, dc*128+p]   (fp16)
    xt = nc.declare_dram_parameter("xt", [BSH, NQ, 128, 8, TQ], F16, isOutput=False)
    w1s = nc.declare_dram_parameter("w1s", [128, 8, H], F16, isOutput=False)
    b1s = nc.declare_dram_parameter("b1s", [128, 2], F32, isOutput=False)
    w2s = nc.declare_dram_parameter("w2s", [128, 2], F16, isOutput=False)
    out = nc.declare_dram_parameter("out", [BSH, 8, 128], F32, isOutput=True)

    with tile.TileContext(nc) as tc:
        with tc.tile_pool(name="w", bufs=1) as wpool, \
             tc.tile_pool(name="x", bufs=XBUFS) as xpool, \
             tc.tile_pool(name="h", bufs=4) as hpool, \
             tc.tile_pool(name="e", bufs=2) as epool, \
             tc.tile_pool(name="bis", bufs=1) as bpool, \
             tc.tile_pool(name="bc", bufs=BCBUFS) as bcpool, \
             tc.tile_pool(name="p2", bufs=2) as p2pool, \
             tc.tile_pool(name="ps", bufs=6, space="PSUM") as pspool, \
             tc.tile_pool(name="pse", bufs=2, space="PSUM") as psepool, \
             tc.tile_pool(name="dram", bufs=1, space="DRAM") as dpool:

            e_dram = dpool.tile([BSH, T], F32, tag="e_dram")
            bt_dram = dpool.tile([BSH, T], F16, tag="bt_dram")

            w1sb = wpool.tile([128, 8, H], F16, tag="w1sb")
            nc.sync.dma_start(w1sb[:], w1s.ap())
            b1sb = wpool.tile([128, 2], F32, tag="b1sb")
            nc.sync.dma_start(b1sb[:], b1s.ap())
            w2sb = wpool.tile([128, 2], F16, tag="w2sb")
            nc.sync.dma_start(w2sb[:], w2s.ap())

            nbig = wpool.tile([32, 128], F32, tag="nbig")
            nc.vector.memset(nbig[:], NEG_BIG)
            # valid[p] = 1 on partitions {0,1,2} (the 3 candidate slots)
            valid = wpool.tile([32, 1], F32, tag="valid")
            nc.vector.memset(valid[:], 0.0)
            nc.vector.memset(valid[0:3, :], 1.0)
            qvec = wpool.tile([32, 3], F32, tag="qvec")
            for ci in range(3):
                nc.vector.memset(qvec[:, ci : ci + 1], float(ci + 1))

            for _rep in range(repeat):
                xts = {}

                def load(b):
                    for q in range(NQ):
                        xq = xpool.tile([128, 8, TQ], F16, tag="x")
                        xts[(b, q)] = xq
                        for hf in range(2):
                            sl = slice(hf * (TQ // 2), (hf + 1) * (TQ // 2))
                            nc.sync.dma_start(xq[:, :, sl], xt.ap()[b, q, :, :, sl])

                def p1(b, weave=()):
                    weave = list(weave)
                    for ti in range(T // TT):
                        q, half = divmod(ti, 2)
                        sl = slice(half * TT, half * TT + TT)
                        xq = xts[(b, q)]
                        hs = []
                        for hh in range(2):
                            hsl = slice(hh * 128, hh * 128 + 128)
                            ps = pspool.tile([128, TT], F32, tag="hps")
                            for dc in range(8):
                                nc.tensor.matmul(
                                    ps[:], w1sb[:, dc, hsl], xq[:, dc, sl],
                                    start=(dc == 0), stop=(dc == 7),
                                )
                            hsb = hpool.tile([128, TT], F16, tag="h")
                            nc.scalar.activation(
                                hsb[:], ps[:], AF.Tanh, bias=b1sb[:, hh : hh + 1]
                            )
                            hs.append(hsb)
                        eps = psepool.tile([1, TT], F32, tag="eps")
                        nc.tensor.matmul(eps[:], w2sb[:, 0:1], hs[0][:], start=True, stop=False)
                        nc.tensor.matmul(eps[:], w2sb[:, 1:2], hs[1][:], start=False, stop=True)
                        estage = epool.tile([1, TT], F32, tag="estage")
                        nc.scalar.copy(estage[:], eps[:])
                        sl_d = slice(ti * TT, ti * TT + TT)
                        nc.sync.dma_start(e_dram[b : b + 1, sl_d], estage[:])
                        # up to 5 woven emissions per tile gap
                        for _ in range(9):
                            if weave:
                                weave.pop(0)()
                    while weave:
                        weave.pop(0)()

                def bisect(b):
                    """Exact 4-ary bisection for the 2048th order statistic of
                    sample b's e on a [32,128] relayout. Returns (Eb, lo)."""
                    if PHASE < 2:
                        return None
                    Eb = bpool.tile([32, 128], F32, tag="Eb", name=f"Eb{b}")
                    nc.sync.dma_start(
                        Eb[:], e_dram[b].rearrange("(lp f) -> lp f", lp=32)
                    )
                    lo_a = bpool.tile([32, 1], F32, tag="lo_a", name=f"lo_a{b}")
                    lo_b = bpool.tile([32, 1], F32, tag="lo_b", name=f"lo_b{b}")
                    nc.vector.memset(lo_a[:], -0.25)
                    cj = bpool.tile([32, 128], U8, tag="cj", name=f"cj{b}")
                    cand = bpool.tile([32, 3], F32, tag="cand", name=f"cand{b}")
                    cscr = bpool.tile([32, 3], F32, tag="cscr", name=f"cscr{b}")
                    tot3 = bpool.tile([32, 3], F32, tag="tot3", name=f"tot3{b}")
                    ks3 = bpool.tile([32, 3], F32, tag="ks3", name=f"ks3{b}")
                    kb = bpool.tile([32, 1], F32, tag="kb", name=f"kb{b}")
                    lo, lo2 = lo_a, lo_b
                    w = 0.5
                    for _ in range(N_ITER):
                        q = w / 4.0
                        nc.vector.tensor_scalar(
                            cand[:, 0:3], qvec[:], q, lo[:], ALU.mult, ALU.add
                        )
                        for ci in range(3):
                            nc.vector.tensor_scalar(
                                cj[:], Eb[:], cand[:, ci : ci + 1], 0.0,
                                ALU.is_lt, ALU.add, accum_out=cscr[:, ci : ci + 1],
                            )
                        # per-sample candidate totals, broadcast to all parts
                        nc.gpsimd.partition_all_reduce(
                            tot3[:], cscr[:], channels=32,
                            reduce_op=bass_isa.ReduceOp.add,
                        )
                        nc.vector.tensor_scalar(ks3[:], tot3[:], 2048.5, None, ALU.is_lt)
                        nc.vector.tensor_reduce(kb[:], ks3[:], axis=AX.X, op=ALU.add)
                        nc.vector.tensor_scalar(lo2[:], kb[:], q, lo[:], ALU.mult, ALU.add)
                        lo, lo2 = lo2, lo
                        w = q
                    return Eb, lo

                def softmax(b, Eb, lo):
                    """beta = exp(e - thr) * [e < thr] / Z on [32,128];
                    beta row (fp16) bounces via DRAM."""
                    if PHASE < 3:
                        return
                    tn = bpool.tile([32, 1], F32, tag="tn", name=f"tn{b}")
                    nc.vector.tensor_scalar(tn[:], lo[:], -1.0, None, ALU.mult)
                    msk = bpool.tile([32, 128], U8, tag="msk", name=f"msk{b}")
                    nc.vector.tensor_scalar(msk[:], Eb[:], lo[:], None, ALU.is_ge)
                    nc.vector.copy_predicated(Eb[:], msk[:], nbig[:])
                    zp = bpool.tile([32, 1], F32, tag="zp", name=f"zp{b}")
                    u = bpool.tile([32, 128], F32, tag="u", name=f"u{b}")
                    nc.scalar.activation(
                        u[:], Eb[:], AF.Exp, bias=tn[:, 0:1], scale=1.0,
                        accum_out=zp[:],
                    )
                    zb = bpool.tile([32, 1], F32, tag="zb", name=f"zb{b}")
                    nc.gpsimd.partition_all_reduce(
                        zb[:], zp[:], channels=32, reduce_op=bass_isa.ReduceOp.add
                    )
                    rz = bpool.tile([32, 1], F32, tag="rz", name=f"rz{b}")
                    nc.vector.reciprocal(rz[:], zb[:])
                    beta16 = bpool.tile([32, 128], F16, tag="beta16", name=f"b16{b}")
                    nc.vector.tensor_scalar(beta16[:], u[:], rz[:], None, ALU.mult)
                    nc.sync.dma_start(
                        bt_dram[b].rearrange("(lp f) -> lp f", lp=32), beta16[:]
                    )

                def p2_thunks(b):
                    """Emission thunks: beta broadcast + 32 (dc, q) chunks +
                    accumulator fold + out DMA."""
                    if PHASE < 4:
                        return []
                    thunks = []
                    st = {}

                    def mk_bcast():
                        brow = p2pool.tile([1, T], F16, tag="brow", name=f"brow{b}", bufs=1)
                        nc.sync.dma_start(brow[:], bt_dram[b : b + 1, :])
                        bc = bcpool.tile([128, T], F16, tag="bc")
                        for sq in range(8):
                            ssl = slice(sq * 512, (sq + 1) * 512)
                            nc.gpsimd.partition_broadcast(
                                bc[:, ssl], brow[:, ssl], channels=128
                            )
                        acc4 = p2pool.tile([128, 8 * NQ], F32, tag="acc4", name=f"acc4_{b}")
                        st["bc"] = bc
                        st["acc4"] = acc4
                    thunks.append(mk_bcast)

                    def mk_chunk(dc, q):
                        def f():
                            bc, acc4 = st["bc"], st["acc4"]
                            xq = xts[(b, q)]
                            in0 = xq[:, dc, :]
                            in1 = bc[:, q * TQ : (q + 1) * TQ]
                            a_out = acc4[:, dc * NQ + q : dc * NQ + q + 1]
                            eng = ENGMAP[b][dc]
                            if eng == "B":
                                wm = p2pool.tile([128, TQ], F16, tag="wm")
                                nc.vector.tensor_tensor(
                                    out=wm[:], in0=in0, in1=in1, op=ALU.mult
                                )
                                jk = p2pool.tile([128, TQ], F16, tag="jkA", bufs=1)
                                nc.scalar.activation(
                                    jk[:], wm[:], AF.Copy, accum_out=a_out
                                )
                            elif eng == "C":
                                wp = p2pool.tile([128, TQ], F16, tag="wp")
                                nc.gpsimd.tensor_tensor(
                                    out=wp[:], in0=in0, in1=in1, op=ALU.mult
                                )
                                jk = p2pool.tile([128, TQ], F16, tag="jkC", bufs=1)
                                nc.scalar.activation(
                                    jk[:], wp[:], AF.Copy, accum_out=a_out
                                )
                            elif eng == "D":
                                wp = p2pool.tile([128, TQ], F16, tag="wp")
                                nc.gpsimd.tensor_tensor(
                                    out=wp[:], in0=in0, in1=in1, op=ALU.mult
                                )
                                nc.vector.tensor_reduce(
                                    a_out, wp[:], axis=AX.X, op=ALU.add
                                )
                            else:  # E: DVE multiply + DVE reduce
                                wm = p2pool.tile([128, TQ], F16, tag="wm")
                                nc.vector.tensor_tensor(
                                    out=wm[:], in0=in0, in1=in1, op=ALU.mult
                                )
                                nc.vector.tensor_reduce(
                                    a_out, wm[:], axis=AX.X, op=ALU.add
                                )
                        return f

                    # ti-major: x tile (b, q) is released as soon as all its
                    # dc chunks ran, matching the load order
                    order = sorted(
                        [(dc, q) for dc in range(8) for q in range(NQ)],
                        key=lambda t: (t[1], {"C": 0, "D": 1, "B": 2, "E": 3}[ENGMAP[b][t[0]]]),
                    )
                    for dc, q in order:
                        thunks.append(mk_chunk(dc, q))

                    def mk_fold():
                        acc4 = st["acc4"]
                        acc8 = p2pool.tile([128, 8], F32, tag="acc8", name=f"acc8_{b}")
                        nc.vector.tensor_reduce(
                            acc8[:], acc4[:].rearrange("p (dc q) -> p dc q", dc=8),
                            axis=AX.X, op=ALU.add,
                        )
                        for dc in range(8):
                            nc.sync.dma_start(out.ap()[b, dc, :], acc8[:, dc : dc + 1])
                    thunks.append(mk_fold)
                    return thunks

                def post(b):
                    r = bisect(b)
                    if r is not None:
                        softmax(b, *r)

                load(0)
                load(1)
                p1(0)
                post(0)
                p1(1, weave=p2_thunks(0))
                load(2)
                post(1)
                p1(2, weave=p2_thunks(1))
                load(3)
                post(2)
                p1(3, weave=p2_thunks(2))
                post(3)
                for th in p2_thunks(3):
                    th()
                if PHASE < 4:
                    zt0 = p2pool.tile([128, 8], F32, tag="ztf")
                    nc.vector.memset(zt0[:], float(PHASE))
                    for b in range(BSH):
                        for dc in range(8):
                            nc.sync.dma_start(out.ap()[b, dc, :], zt0[:, dc : dc + 1])

    nc.finalize()
    return nc


_NC_CACHE = None


def _get_nc():
    global _NC_CACHE
    if _NC_CACHE is None:
        _NC_CACHE = build()
    return _NC_CACHE


def make_in_maps(x, W1, b1, W2, b2):
    del b2  # shift-invariant: no effect on the output
    x = np.asarray(x, dtype=np.float32)
    W1 = np.asarray(W1, dtype=np.float32)
    b1 = np.asarray(b1, dtype=np.float32).reshape(H)
    W2 = np.asarray(W2, dtype=np.float32).reshape(H)

    w1s = np.ascontiguousarray(
        W1.reshape(8, 128, H).transpose(1, 0, 2)
    ).astype(np.float16)
    b1s = np.ascontiguousarray(b1.reshape(2, 128).T)
    w2s = np.ascontiguousarray(W2.reshape(2, 128).T).astype(np.float16)

    in_maps = []
    for c in range(8):
        xs = x[4 * c : 4 * c + 4]  # [4, T, D]
        # [b, q, p, dc, tq]: x[b, q*TQ+tq, dc*128+p]
        xtv = (
            xs.transpose(0, 2, 1)
            .reshape(BSH, 8, 128, NQ, TQ)
            .transpose(0, 3, 2, 1, 4)
        )
        in_maps.append(
            {
                "xt": np.ascontiguousarray(xtv).astype(np.float16),
                "w1s": w1s,
                "b1s": b1s,
                "w2s": w2s,
            }
        )
    return in_maps


def kernel(x, W1, b1, W2, b2):
    nc = _get_nc()
    in_maps = make_in_maps(x, W1, b1, W2, b2)
    res = run_bass_kernel_spmd(nc, in_maps, core_ids=list(range(8)))
    outs = [res.results[c]["out"].reshape(BSH, 1024) for c in range(8)]
    full = np.concatenate(outs, axis=0).astype(np.float32)  # [32, 1024]
    return full[:, :, None, None]
